# revision 1
# baseline (speedup 1.0000x reference)
"""BinarySelfAttention Trainium2 kernel (8-core SPMD).

Strategy: shard (batch, head-group): core c -> batch c//4, heads 4*(c%4)..+3.
Each core computes ternary-projected QKV for its 4 heads, RoPE, causal
flash-style attention in S^T orientation (keys on partitions -> no transposes),
and a partial output projection against its Wo column slice. Host sums the 4
partials per batch.

All matmuls run in float32r (TF32-like, full PE rate at moving-dim >= 256).
Ternary weight signs {-1,0,1} are exact in f32r; ternary scales are folded
into the exp() scale (sq*sk/8) and the final output eviction (sv*so), both
passed as runtime data so the compiled program is input-independent.
"""
import numpy as np

import concourse.bass as bass
import concourse.mybir as mybir
import concourse.tile as tile
from concourse.bass_utils import run_bass_kernel_spmd
from concourse.tile_rust import add_dep_helper

F32 = mybir.dt.float32
F32R = mybir.dt.float32r

B, T, D, H = 2, 2048, 1024, 16
HD = 64            # head dim
HPC = 4            # heads per core
FPC = HPC * HD     # features per core (256)
NCORES = 8
KC = D // 128      # 8 contraction chunks for projections


def _split_excess_waits(nc, max_waits=1):
    """TRN2 ISA has one sem-wait slot per instruction and this walrus build
    rejects 3+; hoist excess waits onto preceding same-engine NOPs."""
    n = 0
    for f in nc.m.functions:
        for bb in f.blocks:
            new_insts = []
            for inst in bb.instructions:
                si = getattr(inst, 'sync_info', None)
                if si is not None and si.on_wait and len(si.on_wait) > max_waits:
                    waits = list(si.on_wait)
                    extra, keep = waits[:-max_waits], waits[-max_waits:]
                    for j, w in enumerate(extra):
                        new_insts.append(mybir.InstNoOp(
                            name=f"{inst.name}-wsplit{j}",
                            engine=inst.engine,
                            sync_info=mybir.SyncInfo(on_wait=[w], on_update=[]),
                            bass_nofuse=True,
                        ))
                        n += 1
                    inst.sync_info = mybir.SyncInfo(
                        on_wait=keep, on_update=si.on_update)
                new_insts.append(inst)
            bb.instructions[:] = new_insts
    return n


def _build():
    nc = bass.Bass("TRN2", target_bir_lowering=False, debug=False,
                   num_devices=NCORES)
    xt_d = nc.dram_tensor("xt", [D, T], F32R, kind="ExternalInput")
    wq_d = nc.dram_tensor("wqt", [D, FPC], F32R, kind="ExternalInput")
    wk_d = nc.dram_tensor("wkt", [D, FPC], F32R, kind="ExternalInput")
    wv_d = nc.dram_tensor("wvt", [D, FPC], F32R, kind="ExternalInput")
    wo_d = nc.dram_tensor("woc", [FPC, D], F32R, kind="ExternalInput")
    cos_d = nc.dram_tensor("cos2", [128, T], F32, kind="ExternalInput")
    sin_d = nc.dram_tensor("sins", [128, T], F32, kind="ExternalInput")
    msk_d = nc.dram_tensor("maskm", [128, 128], F32R, kind="ExternalInput")
    con_d = nc.dram_tensor("consts", [128, 2], F32, kind="ExternalInput")
    yp_d = nc.dram_tensor("yp", [T, D], F32, kind="ExternalOutput")
    rec_d = nc.dram_tensor("recd", [HPC, T], F32)  # internal scratch

    EXP = mybir.ActivationFunctionType.Exp
    CPY = mybir.ActivationFunctionType.Copy

    with tile.TileContext(nc) as tc:
        with tc.tile_pool(name="main", bufs=1) as mp:
            CON = mp.tile([128, 2], F32)
            MSK = mp.tile([128, 128], F32R)
            QT = [mp.tile([128, T], F32R, tag=f"qt{i}", name=f"qt{i}") for i in range(2)]
            KT = [mp.tile([128, T], F32R, tag=f"kt{i}", name=f"kt{i}") for i in range(2)]
            VA = mp.tile([128, 16, HPC * 65], F32R)
            ONES = mp.tile([128, 64], F32)

            nc.sync.dma_start(out=CON, in_=con_d[:, :])
            nc.sync.dma_start(out=MSK, in_=msk_d[:, :])
            nc.vector.memset(ONES, 1.0)
            ones_view = VA[:, :, :].rearrange(
                "p a (h e) -> p a h e", e=65)[:, :, :, 64:65].rearrange(
                "p a h e -> p (a h e)")
            nc.vector.tensor_copy(out=ones_view, in_=ONES[:, 0:64])

            # ---------------- Phase 1: projections + RoPE ----------------
            ptp_cm = tc.tile_pool(name="pt", bufs=7)
            ptp = ptp_cm.__enter__()
            with tc.tile_pool(name="p1", bufs=1) as p1, \
                 tc.tile_pool(name="wp", bufs=3) as wp:
                XT = p1.tile([128, KC, T], F32R)
                COS = p1.tile([128, T], F32)
                SIN = p1.tile([128, T], F32)

                _engs = [nc.sync, nc.scalar, nc.gpsimd]

                # kc-major interleave: weight chunk then its x chunks, so
                # the kc-streaming Q projection consumes data on arrival
                wts = {}
                for wname in ("q", "k", "v"):
                    wts[wname] = wp.tile([128, KC, FPC], F32R, tag="w",
                                         name=f"wt_{wname}")
                for kc in range(KC):
                    nc.sync.dma_start(
                        out=wts["q"][:, kc, :],
                        in_=wq_d[128 * kc:128 * kc + 128, :])
                    nc.gpsimd.dma_start(
                        out=wts["k"][:, kc, :],
                        in_=wk_d[128 * kc:128 * kc + 128, :])
                    for tch in range(4):
                        eng = nc.sync if tch % 2 == 0 else nc.scalar
                        eng.dma_start(
                            out=XT[:, kc, 512 * tch:512 * tch + 512],
                            in_=xt_d[128 * kc:128 * kc + 128,
                                     512 * tch:512 * tch + 512])
                for kc in range(KC):
                    nc.gpsimd.dma_start(
                        out=wts["v"][:, kc, :],
                        in_=wv_d[128 * kc:128 * kc + 128, :])
                nc.scalar.dma_start(out=COS, in_=cos_d[:, :])
                nc.scalar.dma_start(out=SIN, in_=sin_d[:, :])

                def proj_qk(wt, dest, evict_eng, psqk, pfx):
                    # kc-streaming: 8 persistent accumulators (8 PSUM banks)
                    accs = [psqk.tile([128, 512], F32, tag=f"pq{i}",
                                      name=f"{pfx}acc{i}") for i in range(8)]
                    for kc in range(KC):
                        for dt_i in range(2):
                            for tch in range(4):
                                nc.tensor.matmul(
                                    accs[4 * dt_i + tch],
                                    wt[:, kc, 128 * dt_i:128 * dt_i + 128],
                                    XT[:, kc, 512 * tch:512 * tch + 512],
                                    start=(kc == 0), stop=(kc == KC - 1))
                    for dt_i in range(2):
                        for tch in range(4):
                            eng = (nc.vector.tensor_copy if tch % 2 == 0
                                   else nc.scalar.copy)
                            eng(
                                out=dest[dt_i][:, 512 * tch:512 * tch + 512],
                                in_=accs[4 * dt_i + tch])

                def rope(dest, pfx):
                    # in-place rope on the f32r projection output
                    for dt_i in range(2):
                        dst = dest[dt_i]
                        rot = p1.tile([128, T], F32R, tag=f"rot{dt_i}",
                                      name=f"{pfx}rot{dt_i}")
                        for g in range(2):
                            b0 = 64 * g
                            nc.gpsimd.dma_start(out=rot[b0:b0 + 32, :],
                                                in_=dst[b0 + 32:b0 + 64, :])
                            nc.gpsimd.dma_start(out=rot[b0 + 32:b0 + 64, :],
                                                in_=dst[b0:b0 + 32, :])
                        nc.gpsimd.tensor_mul(rot, rot, SIN)
                        nc.vector.tensor_mul(dst, dst, COS)
                        nc.vector.tensor_add(dst, dst, rot)

                with tc.tile_pool(name="psqk", bufs=1,
                                  space="PSUM") as psqk:
                    proj_qk(wts["q"], QT, nc.vector.tensor_copy, psqk, "q")
                    proj_qk(wts["k"], KT, nc.vector.tensor_copy, psqk, "k")
                    rope(QT, "q")
                    rope(KT, "k")

                # attention pools open early: S/exp for (h0,qh0) is
                # prefetched before the V projection to hide V evictions
                pss_cm = tc.tile_pool(name="pss", bufs=2, space="PSUM")
                pss = pss_cm.__enter__()

                def s_exp_piece(h, qh, kc):
                    qt, kt = QT[h // 2], KT[h // 2]
                    r0 = 64 * (h % 2)
                    q0, q1 = 1024 * qh, 1024 * qh + 1024
                    qs = max(q0, 128 * kc)
                    cols = q1 - qs
                    sp = pss.tile([128, 1024], F32, tag="sp")
                    off = 0
                    while off < cols:
                        # a matmul must not cross a 512-f32 PSUM bank edge
                        cw = min(512 - (off % 512), cols - off)
                        nc.tensor.matmul(
                            sp[:, off:off + cw],
                            kt[r0:r0 + 64, 128 * kc:128 * kc + 128],
                            qt[r0:r0 + 64, qs + off:qs + off + cw],
                            start=True, stop=True)
                        off += cw
                    pt = ptp.tile([128, 1024], F32R, tag="pt")
                    nc.scalar.activation(
                        out=pt[:, 0:cols], in_=sp[:, 0:cols],
                        func=EXP, scale=CON[:, 0:1])
                    if 128 * kc >= q0:  # diagonal block leads piece
                        nc.vector.tensor_mul(
                            pt[:, 0:128], pt[:, 0:128], MSK)
                    return pt, qs, cols

                def pv_piece(yaug, h, qh, kc, pt, qs, cols):
                    q0 = 1024 * qh
                    off = 0
                    while off < cols:
                        # PV chunks aligned to 512-windows so each window's
                        # PSUM accumulation group is clean
                        cw = min(512 - ((qs + off) % 512), cols - off)
                        w = (qs + off) // 512
                        nc.tensor.matmul(
                            yaug[:, qs - q0 + off:qs - q0 + off + cw],
                            VA[:, kc, 65 * h:65 * h + 65],
                            pt[:, off:off + cw],
                            start=(kc == 0), stop=(kc == 4 * w + 3))
                        off += cw

                pre_pts = [s_exp_piece(0, 0, kc) for kc in range(4)]

                # V projection -> VA [keys, 4*(64+ones)]
                wtv = wts["v"]
                with tc.tile_pool(name="psv", bufs=4, space="PSUM") as psv:
                    for t16 in range(16):
                        acc = psv.tile([128, FPC], F32, tag="pv")
                        for kc in range(KC):
                            nc.tensor.matmul(
                                acc,
                                XT[:, kc, 128 * t16:128 * t16 + 128],
                                wtv[:, kc, :],
                                start=(kc == 0), stop=(kc == KC - 1))
                        veng = (nc.vector.tensor_copy if t16 % 2 == 0
                                else nc.scalar.copy)
                        veng(
                            out=VA[:, t16, :].rearrange(
                                "p (h e) -> p h e", e=65)[:, :, 0:64],
                            in_=acc.rearrange("p (h e) -> p h e", e=64))

            # ------- Phase 2: attention, q-halved for tail overlap -------
            atp_cm = tc.tile_pool(name="atp", bufs=1)
            atp = atp_cm.__enter__()
            AT = [atp.tile([128, T], F32R, tag=f"at{i}", name=f"at{i}")
                  for i in range(2)]
            WOC = atp.tile([128, 2, D], F32R)
            for ft in range(2):
                nc.sync.dma_start(out=WOC[:, ft, :],
                                  in_=wo_d[128 * ft:128 * ft + 128, :])
            with tc.tile_pool(name="p2", bufs=2) as p2, \
                 tc.tile_pool(name="rb", bufs=2) as rbp, \
                 tc.tile_pool(name="psy", bufs=2, space="PSUM") as psy:
                for h in (0, 1, 3, 2):
                    for qh in range(2):  # q half: [1024*qh, 1024*qh+1024)
                        q0, q1 = 1024 * qh, 1024 * qh + 1024
                        yaug = psy.tile([65, 1024], F32, tag="yaug")
                        for kc in range(8 * (qh + 1)):
                            if h == 0 and qh == 0 and kc < 4:
                                pt, qs, cols = pre_pts[kc]
                            else:
                                pt, qs, cols = s_exp_piece(h, qh, kc)
                            pv_piece(yaug, h, qh, kc, pt, qs, cols)
                        rec = p2.tile([1, 1024], F32, tag="rec")
                        nc.vector.reciprocal(out=rec, in_=yaug[64:65, :])
                        wr_i = nc.sync.dma_start(out=rec_d[h, q0:q1],
                                                 in_=rec)
                        rb = rbp.tile([64, 1024], F32, tag="rb")
                        rsrc = rec_d[h, q0:q1]
                        rd_i = nc.sync.dma_start(
                            out=rb,
                            in_=bass.AP(tensor=rsrc.tensor,
                                        offset=rsrc.offset,
                                        ap=[[0, 64]] + list(rsrc.ap)))
                        # Tile does not track DRAM scratch RAW deps
                        add_dep_helper(rd_i.ins, wr_i.ins, sync=True,
                                       reason="recd bounce RAW")
                        if h % 2 == 0:
                            nc.vector.tensor_mul(
                                AT[h // 2][0:64, q0:q1], yaug[0:64, :], rb)
                        else:
                            stg = p2.tile([64, 1024], F32R, tag="stg")
                            nc.vector.tensor_mul(stg, yaug[0:64, :], rb)
                            nc.sync.dma_start(
                                out=AT[h // 2][64:128, q0:q1], in_=stg)

            # ---------------- Phase 3: output projection ----------------
            with tc.tile_pool(name="p3", bufs=3) as p3, \
                 tc.tile_pool(name="pso", bufs=2, space="PSUM") as pso:
                for t16 in range(16):
                    yo = pso.tile([128, D], F32, tag="yo")
                    for half in range(2):
                        for ft in range(2):
                            nc.tensor.matmul(
                                yo[:, 512 * half:512 * half + 512],
                                AT[ft][:, 128 * t16:128 * t16 + 128],
                                WOC[:, ft, 512 * half:512 * half + 512],
                                start=(ft == 0), stop=(ft == 1))
                    ot = p3.tile([128, D], F32, tag="ot")
                    nc.scalar.activation(out=ot, in_=yo, func=CPY,
                                         scale=CON[:, 1:2])
                    nc.sync.dma_start(
                        out=yp_d[128 * t16:128 * t16 + 128, :], in_=ot)
            atp_cm.__exit__(None, None, None)
            ptp_cm.__exit__(None, None, None)
            pss_cm.__exit__(None, None, None)

    _split_excess_waits(nc)
    return nc


_NC = None
_LAST_INMAPS = None


def _get_nc():
    global _NC
    if _NC is None:
        _NC = _build()
    return _NC


def _ternary_signs(w):
    """Mirror reference ternary_weight: returns (signs in {-1,0,1}, scale)."""
    try:
        import jax
        import jax.numpy as jnp
        cpu = jax.devices("cpu")[0]
        with jax.default_device(cpu):
            wj = jnp.asarray(np.asarray(w, dtype=np.float32))
            scale = jnp.mean(jnp.abs(wj))
            signs = jnp.round(jnp.clip(wj / (scale + 1e-8), -1.0, 1.0))
            return np.asarray(signs, dtype=np.float32), float(scale)
    except Exception:
        w = np.asarray(w, dtype=np.float32)
        scale = np.float32(np.mean(np.abs(w)))
        signs = np.round(np.clip(w / (scale + np.float32(1e-8)), -1.0, 1.0))
        return signs.astype(np.float32), float(scale)


def _round12(a):
    """Round fp32 to 12 mantissa bits (representable in f32r)."""
    u = np.ascontiguousarray(a, dtype=np.float32).view(np.uint32)
    r = (u + np.uint32(1 << 10)) & np.uint32(0xFFFFF800)
    return r.view(np.float32)


def _rope_tables():
    inv = (1.0 / (10000.0 ** (np.arange(0, HD, 2, dtype=np.float32) / HD))
           ).astype(np.float32)                      # [32]
    t = np.arange(T, dtype=np.float32)
    fr = np.outer(t, inv).astype(np.float32)         # [T, 32]
    cos1 = np.cos(fr).astype(np.float32)             # [T, 32]
    sin1 = np.sin(fr).astype(np.float32)
    # rows: d in 0..63 (freq d%32), tiled for 2 heads -> 128 rows
    cosd = np.concatenate([cos1, cos1], axis=1).T    # [64, T]
    sind = np.concatenate([sin1, sin1], axis=1).T    # [64, T]
    sgn = np.ones((HD, 1), dtype=np.float32)
    sgn[:HD // 2] = -1.0
    cos2 = np.tile(cosd, (2, 1)).astype(np.float32)          # [128, T]
    sins = np.tile(sind * sgn, (2, 1)).astype(np.float32)    # [128, T]
    return cos2, sins


def kernel(x, Wq, Wk, Wv, Wo, mask):
    global _LAST_INMAPS
    x = np.asarray(x, dtype=np.float32)
    mask = np.asarray(mask)
    assert np.array_equal(
        np.asarray(mask[0, 0], dtype=np.int32),
        np.tril(np.ones((T, T), dtype=np.int32))), "non-causal mask"

    qs, sq = _ternary_signs(Wq)
    ks, sk = _ternary_signs(Wk)
    vs, sv = _ternary_signs(Wv)
    os_, so = _ternary_signs(Wo)
    cos2, sins = _rope_tables()
    mvals = np.triu(np.ones((128, 128), dtype=np.float32))  # valid: i <= j
    consts = np.zeros((128, 2), dtype=np.float32)
    consts[:, 0] = np.float32(sq) * np.float32(sk) * np.float32(0.125)
    consts[:, 1] = np.float32(sv) * np.float32(so)

    in_maps = []
    for c in range(NCORES):
        b, g = c // 4, c % 4
        fsl = slice(FPC * g, FPC * g + FPC)
        in_maps.append({
            "xt": _round12(x[b].T),
            "wqt": np.ascontiguousarray(qs[fsl].T),
            "wkt": np.ascontiguousarray(ks[fsl].T),
            "wvt": np.ascontiguousarray(vs[fsl].T),
            "woc": np.ascontiguousarray(os_[:, fsl].T),
            "cos2": cos2,
            "sins": sins,
            "maskm": mvals,
            "consts": consts,
        })
    _LAST_INMAPS = in_maps

    res = run_bass_kernel_spmd(_get_nc(), in_maps,
                               core_ids=list(range(NCORES)))
    out = np.zeros((B, T, D), dtype=np.float32)
    for b in range(B):
        acc = np.zeros((T, D), dtype=np.float32)
        for g in range(4):
            acc += res.results[4 * b + g]["yp"]
        out[b] = acc
    return out


def bench(trace=True):
    """Re-run last inputs with NTFF tracing; returns BassKernelResults."""
    assert _LAST_INMAPS is not None, "call kernel() first"
    return run_bass_kernel_spmd(_get_nc(), _LAST_INMAPS,
                                core_ids=list(range(NCORES)), trace=trace)



# revision 41
# speedup vs baseline: 1.2786x; 1.2786x over previous
"""BinarySelfAttention Trainium2 kernel (8-core SPMD).

Strategy: shard (batch, head-group): core c -> batch c//4, heads 4*(c%4)..+3.
Each core computes ternary-projected QKV for its 4 heads, RoPE, causal
flash-style attention in S^T orientation (keys on partitions -> no transposes),
and a partial output projection against its Wo column slice. Host sums the 4
partials per batch.

Precision plan (cost model: bf16 matmul = 1 cycle/row at any width; fp8e4 +
DoubleRow = 0.5 cycles/row; f32r pays 4x on <256-wide chunks):
- Q/K projections: fp8e4 x and ternary signs, DoubleRow perf mode (256-deep
  contraction per instruction). Scores only shift ~1% from fp8 x.
- Everything else (V proj, S, PV, out proj): bf16 operands, f32 PSUM.
- Ternary scales fold into the exp() scale (sq*sk/8, runtime data) and into
  the host-prepared Wo slice (sv*so), keeping the program input-independent.

Schedule: PE stream is software-pipelined (S piece kc+1 issued before PV of
kc so exp latency hides behind matmuls); rope of head-pair 1 is deferred into
head 0/1's attention window; reciprocal broadcasts bounce through DRAM in
head pairs (one write + one read per pair).
"""
import numpy as np

import concourse.bass as bass
import concourse.mybir as mybir
import concourse.tile as tile
from concourse.bass_utils import run_bass_kernel_spmd
from concourse.tile_rust import add_dep_helper

F32 = mybir.dt.float32
BF16 = mybir.dt.bfloat16
FP8 = mybir.dt.float8e4
DR = mybir.MatmulPerfMode.DoubleRow

B, T, D, H = 2, 2048, 1024, 16
HD = 64            # head dim
HPC = 4            # heads per core
FPC = HPC * HD     # features per core (256)
NCORES = 8
KC = D // 128      # 8 contraction chunks of 128


def _split_excess_waits(nc, max_waits=1):
    """TRN2 ISA has one sem-wait slot per instruction and this walrus build
    rejects 3+; hoist excess waits onto preceding same-engine NOPs."""
    n = 0
    for f in nc.m.functions:
        for bb in f.blocks:
            new_insts = []
            for inst in bb.instructions:
                si = getattr(inst, 'sync_info', None)
                if si is not None and si.on_wait and len(si.on_wait) > max_waits:
                    waits = list(si.on_wait)
                    extra, keep = waits[:-max_waits], waits[-max_waits:]
                    for j, w in enumerate(extra):
                        new_insts.append(mybir.InstNoOp(
                            name=f"{inst.name}-wsplit{j}",
                            engine=inst.engine,
                            sync_info=mybir.SyncInfo(on_wait=[w], on_update=[]),
                            bass_nofuse=True,
                        ))
                        n += 1
                    inst.sync_info = mybir.SyncInfo(
                        on_wait=keep, on_update=si.on_update)
                new_insts.append(inst)
            bb.instructions[:] = new_insts
    return n


def _build():
    nc = bass.Bass("TRN2", target_bir_lowering=False, debug=False,
                   num_devices=NCORES)
    x8_d = nc.dram_tensor("x8", [D, T], FP8, kind="ExternalInput")
    xb_d = nc.dram_tensor("xb", [D, T], BF16, kind="ExternalInput")
    wq_d = nc.dram_tensor("wq8", [128, 8 * FPC], FP8, kind="ExternalInput")
    wk_d = nc.dram_tensor("wk8", [128, 8 * FPC], FP8, kind="ExternalInput")
    wv_d = nc.dram_tensor("wvb", [128, KC, FPC], BF16, kind="ExternalInput")
    wo_d = nc.dram_tensor("wob", [128, 2, D], BF16, kind="ExternalInput")
    cos_d = nc.dram_tensor("cosb", [128, T], BF16, kind="ExternalInput")
    sin_d = nc.dram_tensor("sinb", [128, T], BF16, kind="ExternalInput")
    msk_d = nc.dram_tensor("mskb", [128, 128], BF16, kind="ExternalInput")
    con_d = nc.dram_tensor("conf", [128, 1], F32, kind="ExternalInput")
    yp_d = nc.dram_tensor("yp", [T, D], BF16, kind="ExternalOutput")
    rec_d = nc.dram_tensor("recd", [2, 2, T], BF16)  # internal scratch

    EXP = mybir.ActivationFunctionType.Exp

    with tile.TileContext(nc) as tc:
        with tc.tile_pool(name="main", bufs=1) as mp:
            X8 = mp.tile([128, KC, T], FP8)
            XB = mp.tile([128, KC, T], BF16)
            W8Q = mp.tile([128, 4, 2, FPC], FP8)
            W8K = mp.tile([128, 4, 2, FPC], FP8)
            WV = mp.tile([128, KC, FPC], BF16)
            WOC = mp.tile([128, 2, D], BF16)
            COS = mp.tile([128, T], BF16)
            SIN = mp.tile([128, T], BF16)
            MSK = mp.tile([128, 128], BF16)
            CON = mp.tile([128, 1], F32)
            QT = [mp.tile([128, T], BF16, tag=f"qt{i}", name=f"qt{i}")
                  for i in range(2)]
            KT = [mp.tile([128, T], BF16, tag=f"kt{i}", name=f"kt{i}")
                  for i in range(2)]
            VA = mp.tile([128, 16, HPC * 65], BF16)
            AT = [mp.tile([128, T], BF16, tag=f"at{i}", name=f"at{i}")
                  for i in range(2)]

            # ---------------- DMA preamble ----------------
            # scalar queue: weights + tables; sync queue: activations.
            # (rot-swap DMAs go on the DVE queue so nothing blocks them.)
            nc.scalar.dma_start(out=W8Q.rearrange("p a b f -> p (a b f)"),
                                in_=wq_d[:, :])
            nc.scalar.dma_start(out=W8K.rearrange("p a b f -> p (a b f)"),
                                in_=wk_d[:, :])
            for kp in range(4):  # x8 in kc pairs (pair 0 split for warmup)
                for (t0, t1) in ([(0, 1024), (1024, T)] if kp == 0
                                 else [(0, T)]):
                    src = x8_d[256 * kp:256 * kp + 256, t0:t1]
                    nc.sync.dma_start(
                        out=X8[:, 2 * kp:2 * kp + 2, t0:t1],
                        in_=bass.AP(tensor=src.tensor, offset=src.offset,
                                    ap=[[T, 128], [128 * T, 2],
                                        [1, t1 - t0]]))
            nc.scalar.dma_start(out=WV, in_=wv_d[:, :, :])
            nc.scalar.dma_start(out=COS, in_=cos_d[:, :])
            nc.scalar.dma_start(out=SIN, in_=sin_d[:, :])
            nc.scalar.dma_start(out=MSK, in_=msk_d[:, :])
            nc.scalar.dma_start(out=CON, in_=con_d[:, :])
            # xb by T-blocks so V projection can stream early.
            # On the scalar queue: the sync queue must stay clear for the
            # rot-swap DMAs that gate attention start.
            for tb in range(4):
                src = xb_d[0:128, 512 * tb:512 * tb + 512]
                nc.scalar.dma_start(
                    out=XB[:, :, 512 * tb:512 * tb + 512],
                    in_=bass.AP(tensor=src.tensor, offset=src.offset,
                                ap=[[T, 128], [128 * T, KC], [1, 512]]))
            nc.scalar.dma_start(out=WOC, in_=wo_d[:, :, :])

            ones_view = VA[:, :, :].rearrange(
                "p a (h e) -> p a h e", e=65)[:, :, :, 64:65].rearrange(
                "p a h e -> p (a h e)")
            nc.vector.memset(ones_view, 1.0)

            # ---------------- Phase 1: QK projections (fp8 DoubleRow) ------
            psqk_cm = tc.tile_pool(name="psqk", bufs=2, space="PSUM")
            psqk = psqk_cm.__enter__()

            def proj_qk(wt, dst, nm):
                for fh in range(2):
                    accs = [psqk.tile([128, 512], F32, tag=f"pa{t}",
                                      name=f"{nm}{fh}a{t}")
                            for t in range(4)]
                    for kcp in range(4):
                        for tch in range(4):
                            nc.tensor.matmul(
                                accs[tch],
                                wt[:, kcp, :, 128 * fh:128 * fh + 128],
                                X8[:, 2 * kcp:2 * kcp + 2,
                                   512 * tch:512 * tch + 512],
                                start=(kcp == 0), stop=(kcp == 3),
                                perf_mode=DR)
                    for tch in range(4):
                        if fh == 0:
                            nc.vector.tensor_copy(
                                out=dst[fh][:, 512 * tch:512 * tch + 512],
                                in_=accs[tch])
                        else:
                            nc.scalar.copy(
                                out=dst[fh][:, 512 * tch:512 * tch + 512],
                                in_=accs[tch])

            proj_qk(W8Q, QT, "q")
            proj_qk(W8K, KT, "k")
            psqk_cm.__exit__(None, None, None)

            # ---------------- RoPE (bf16, DVE; rot swap via DMA) ----------
            rp_cm = tc.tile_pool(name="rp", bufs=2)
            rp = rp_cm.__enter__()

            def rope(dst, pfx, c0=0, c1=T):
                rot = rp.tile([128, T], BF16, tag="rot", name=f"{pfx}rot")
                for g in range(4):
                    b0 = 32 * g
                    s0 = 32 * (g ^ 1)
                    nc.sync.dma_start(out=rot[b0:b0 + 32, c0:c1],
                                      in_=dst[s0:s0 + 32, c0:c1])
                nc.vector.tensor_mul(rot[:, c0:c1], rot[:, c0:c1],
                                     SIN[:, c0:c1])
                nc.vector.tensor_mul(dst[:, c0:c1], dst[:, c0:c1],
                                     COS[:, c0:c1])
                nc.vector.tensor_add(dst[:, c0:c1], dst[:, c0:c1],
                                     rot[:, c0:c1])

            rope(QT[0], "q0")
            rope(KT[0], "k0")

            # ---- Phase 1c/2: V projection + attention (pipelined) -------
            pss_cm = tc.tile_pool(name="pss", bufs=2, space="PSUM")
            pss = pss_cm.__enter__()
            psv_cm = tc.tile_pool(name="psv", bufs=2, space="PSUM")
            psv = psv_cm.__enter__()
            ptp_cm = tc.tile_pool(name="pt", bufs=6)
            ptp = ptp_cm.__enter__()
            pt0_cm = tc.tile_pool(name="pt0", bufs=8)
            pt0 = pt0_cm.__enter__()

            def vproj(t16):
                acc = psv.tile([128, FPC], F32, tag="pv")
                for kc in range(KC):
                    nc.tensor.matmul(
                        acc,
                        XB[:, kc, 128 * t16:128 * t16 + 128],
                        WV[:, kc, :],
                        start=(kc == 0), stop=(kc == KC - 1))
                eng = nc.vector.tensor_copy
                eng(out=VA[:, t16, :].rearrange(
                        "p (h e) -> p h e", e=65)[:, :, 0:64],
                    in_=acc.rearrange("p (h e) -> p h e", e=64))

            _mask_eng = [None]  # None = alternate
            _alt = [0]

            def s_exp_piece(h, qh, kc, pool, ptag):
                qt, kt = QT[h // 2], KT[h // 2]
                r0 = 64 * (h % 2)
                q0, q1 = 1024 * qh, 1024 * qh + 1024
                qs = max(q0, 128 * kc)
                cols = q1 - qs
                sp = pss.tile([128, 1024], F32, tag="sp")
                off = 0
                while off < cols:
                    # matmul must not cross a 512-f32 PSUM bank edge
                    cw = min(512 - (off % 512), cols - off)
                    nc.tensor.matmul(
                        sp[:, off:off + cw],
                        kt[r0:r0 + 64, 128 * kc:128 * kc + 128],
                        qt[r0:r0 + 64, qs + off:qs + off + cw],
                        start=True, stop=True)
                    off += cw
                pt = pool.tile([128, 1024], BF16, tag=ptag)
                nc.scalar.activation(
                    out=pt[:, 0:cols], in_=sp[:, 0:cols],
                    func=EXP, scale=CON[:, 0:1])
                if qs == 128 * kc:  # diagonal block leads piece
                    if _mask_eng[0] is not None:
                        eng = _mask_eng[0]
                    else:
                        _alt[0] += 1
                        eng = (nc.vector.tensor_mul if _alt[0] % 2 == 0
                               else nc.gpsimd.tensor_mul)
                    eng(pt[:, 0:128], pt[:, 0:128], MSK)
                return pt, qs, cols

            def pv_piece(yaug, h, qh, kc, pt, qs, cols):
                q0 = 1024 * qh
                off = 0
                while off < cols:
                    # PV chunks aligned to 512-windows so each window's
                    # PSUM accumulation group is clean
                    cw = min(512 - ((qs + off) % 512), cols - off)
                    w = (qs + off) // 512
                    nc.tensor.matmul(
                        yaug[:, qs - q0 + off:qs - q0 + off + cw],
                        VA[:, kc, 65 * h:65 * h + 65],
                        pt[:, off:off + cw],
                        start=(kc == 0), stop=(kc == 4 * w + 3))
                    off += cw

            # h0/qh0: S+exp now (PV deferred until psv closes)
            _mask_eng[0] = nc.gpsimd.tensor_mul
            vproj(0)
            vproj(1)
            pre_pts = []
            for kc in range(8):
                if kc in (2, 4, 6):
                    vproj(kc)
                pre_pts.append(s_exp_piece(0, 0, kc, pt0, "pt0"))
            for t16 in (3, 5, 7, *range(8, 16)):
                vproj(t16)
            psv_cm.__exit__(None, None, None)

            # ---------------- Phase 2: attention ----------------
            psy_cm = tc.tile_pool(name="psy", bufs=2, space="PSUM")
            psy = psy_cm.__enter__()
            p2_cm = tc.tile_pool(name="p2", bufs=2)
            p2 = p2_cm.__enter__()
            p4_cm = tc.tile_pool(name="p4", bufs=4)
            p4 = p4_cm.__enter__()
            rb_cm = tc.tile_pool(name="rb", bufs=2)
            rbp = rb_cm.__enter__()

            norm_state = {}

            def attention(h, qh):
                """S/PV software pipeline for one (head, query-half)."""
                q0, q1 = 1024 * qh, 1024 * qh + 1024
                yaug = psy.tile([65, 1024], F32, tag="yaug")
                n = 8 * (qh + 1)
                pieces = []
                for kc in range(n):
                    if h == 0 and qh == 0:
                        pieces.append(pre_pts[kc])
                    else:
                        pieces.append(s_exp_piece(h, qh, kc, ptp, "pt"))
                    if kc >= 2:
                        pv_piece(yaug, h, qh, kc - 2, *pieces[kc - 2])
                pv_piece(yaug, h, qh, n - 2, *pieces[n - 2])
                pv_piece(yaug, h, qh, n - 1, *pieces[n - 1])
                # Normalization. Stage-copy yaug to SBUF immediately (frees
                # the PSUM buffer for head h+2); reciprocal rows bounce
                # through DRAM once per head pair for partition broadcast.
                hp = h // 2
                # Single stage-copy (65 rows incl. denominator) releases the
                # yaug PSUM buffer early; reciprocal then reads the bf16
                # stage. qh1 odd heads skip the stage (their 14.5us period
                # hides the bounce latency; the mul reads PSUM directly).
                if qh == 0 or h % 2 == 0:
                    ystg = p4.tile([65, 1024], BF16, tag="ystg")
                    nc.vector.tensor_copy(out=ystg, in_=yaug)
                    den_src, y_src, y_psum = ystg[64:65, :], ystg[0:64, :], 0
                else:
                    den_src, y_src, y_psum = yaug[64:65, :], yaug[0:64, :], 1
                if h % 2 == 0:
                    rec2 = p4.tile([1, 2, 1024], BF16, tag="rec")
                    with nc.allow_low_precision(reason="rec bounce"):
                        nc.vector.reciprocal(out=rec2[:, 0, :], in_=den_src)
                    norm_state[(hp, qh)] = (rec2, y_src)
                    return
                rec2, ysrc_e = norm_state.pop((hp, qh))
                with nc.allow_low_precision(reason="rec bounce"):
                    nc.vector.reciprocal(out=rec2[:, 1, :], in_=den_src)
                wr_i = nc.sync.dma_start(
                    out=rec_d[hp, :, q0:q1],
                    in_=rec2.rearrange("p a c -> p (a c)"))
                rb2 = rbp.tile([64, 2, 1024], BF16, tag="rb")
                rsrc = rec_d[hp, 0, q0:q1]
                rd_i = nc.sync.dma_start(
                    out=rb2,
                    in_=bass.AP(tensor=rsrc.tensor, offset=rsrc.offset,
                                ap=[[0, 64], [T, 2]] + list(rsrc.ap)))
                # Tile does not track DRAM scratch RAW deps
                add_dep_helper(rd_i.ins, wr_i.ins, sync=True,
                               reason="recd bounce RAW")
                # even head -> AT rows 0:64 directly; odd staged + DMA
                nc.gpsimd.tensor_mul(AT[hp][0:64, q0:q1],
                                     ysrc_e, rb2[:, 0, :])
                stg = p2.tile([64, 1024], BF16, tag="stg")
                meng = nc.vector.tensor_mul if y_psum else nc.gpsimd.tensor_mul
                meng(stg, y_src, rb2[:, 1, :])
                nc.sync.dma_start(out=AT[hp][64:128, q0:q1], in_=stg)

            attention(0, 0)
            rope(QT[1], "q1", 0, 1024)
            rope(KT[1], "k1", 0, 1024)
            attention(1, 0)
            attention(2, 0)
            rope(QT[1], "q1b", 1024, T)
            rope(KT[1], "k1b", 1024, T)
            attention(3, 0)
            attention(0, 1)
            attention(1, 1)
            _mask_eng[0] = None
            attention(2, 1)
            attention(3, 1)

            psy_cm.__exit__(None, None, None)
            pss_cm.__exit__(None, None, None)
            rb_cm.__exit__(None, None, None)
            p4_cm.__exit__(None, None, None)
            p2_cm.__exit__(None, None, None)
            pt0_cm.__exit__(None, None, None)
            ptp_cm.__exit__(None, None, None)
            rp_cm.__exit__(None, None, None)

            # ---------------- Phase 3: output projection ----------------
            with tc.tile_pool(name="p3", bufs=4) as p3, \
                 tc.tile_pool(name="pso", bufs=3, space="PSUM") as pso:
                def oproj(t16):
                    yo = pso.tile([128, D], F32, tag="yo")
                    for half in range(2):
                        for ft in range(2):
                            nc.tensor.matmul(
                                yo[:, 512 * half:512 * half + 512],
                                AT[ft][:, 128 * t16:128 * t16 + 128],
                                WOC[:, ft, 512 * half:512 * half + 512],
                                start=(ft == 0), stop=(ft == 1))
                    return yo

                def oevict(t16, yo):
                    ot = p3.tile([128, D], BF16, tag="ot")
                    eng = (nc.scalar.copy, nc.vector.tensor_copy)[t16 % 2]
                    eng(out=ot, in_=yo)
                    deng = nc.sync if t16 % 2 == 0 else nc.scalar
                    deng.dma_start(
                        out=yp_d[128 * t16:128 * t16 + 128, :], in_=ot)

                prev = None
                for t16 in range(16):
                    yo = oproj(t16)
                    if prev is not None:
                        oevict(t16 - 1, prev)
                    prev = yo
                oevict(15, prev)

    _split_excess_waits(nc)
    return nc


_NC = None
_LAST_INMAPS = None


def _get_nc():
    global _NC
    if _NC is None:
        _NC = _build()
    return _NC


def _ternary_signs(w):
    """Mirror reference ternary_weight: returns (signs in {-1,0,1}, scale)."""
    try:
        import jax
        import jax.numpy as jnp
        cpu = jax.devices("cpu")[0]
        with jax.default_device(cpu):
            wj = jnp.asarray(np.asarray(w, dtype=np.float32))
            scale = jnp.mean(jnp.abs(wj))
            signs = jnp.round(jnp.clip(wj / (scale + 1e-8), -1.0, 1.0))
            return np.asarray(signs, dtype=np.float32), float(scale)
    except Exception:
        w = np.asarray(w, dtype=np.float32)
        scale = np.float32(np.mean(np.abs(w)))
        signs = np.round(np.clip(w / (scale + np.float32(1e-8)), -1.0, 1.0))
        return signs.astype(np.float32), float(scale)


def _rope_tables():
    inv = (1.0 / (10000.0 ** (np.arange(0, HD, 2, dtype=np.float32) / HD))
           ).astype(np.float32)                      # [32]
    t = np.arange(T, dtype=np.float32)
    fr = np.outer(t, inv).astype(np.float32)         # [T, 32]
    cos1 = np.cos(fr).astype(np.float32)             # [T, 32]
    sin1 = np.sin(fr).astype(np.float32)
    # rows: d in 0..63 (freq d%32), tiled for 2 heads -> 128 rows
    cosd = np.concatenate([cos1, cos1], axis=1).T    # [64, T]
    sind = np.concatenate([sin1, sin1], axis=1).T    # [64, T]
    sgn = np.ones((HD, 1), dtype=np.float32)
    sgn[:HD // 2] = -1.0
    cos2 = np.tile(cosd, (2, 1)).astype(np.float32)          # [128, T]
    sins = np.tile(sind * sgn, (2, 1)).astype(np.float32)    # [128, T]
    return cos2, sins


def kernel(x, Wq, Wk, Wv, Wo, mask):
    global _LAST_INMAPS
    import ml_dtypes
    F8 = ml_dtypes.float8_e4m3
    BF = ml_dtypes.bfloat16

    x = np.asarray(x, dtype=np.float32)
    mask = np.asarray(mask)
    assert np.array_equal(
        np.asarray(mask[0, 0], dtype=np.int32),
        np.tril(np.ones((T, T), dtype=np.int32))), "non-causal mask"

    qs, sq = _ternary_signs(Wq)
    ks, sk = _ternary_signs(Wk)
    vs, sv = _ternary_signs(Wv)
    os_, so = _ternary_signs(Wo)
    cos2, sins = _rope_tables()
    mvals = np.triu(np.ones((128, 128), dtype=np.float32))  # valid: k <= q
    consts = np.full((128, 1), np.float32(sq) * np.float32(sk) *
                     np.float32(0.125), dtype=np.float32)

    in_maps = []
    for c in range(NCORES):
        b, g = c // 4, c % 4
        fsl = slice(FPC * g, FPC * g + FPC)
        xt = np.ascontiguousarray(x[b].T)            # [D, T]
        # DR weight layout: w8[p, kcp, i, f] = signs[FPC*g+f, 256*kcp+128*i+p]
        wq8 = np.ascontiguousarray(
            qs[fsl].T.reshape(4, 2, 128, FPC).transpose(2, 0, 1, 3)
            ).reshape(128, 8 * FPC)
        wk8 = np.ascontiguousarray(
            ks[fsl].T.reshape(4, 2, 128, FPC).transpose(2, 0, 1, 3)
            ).reshape(128, 8 * FPC)
        wvb = np.ascontiguousarray(
            vs[fsl].T.reshape(KC, 128, FPC).transpose(1, 0, 2))
        wob = np.ascontiguousarray(
            (os_[:, fsl].T * np.float32(sv * so)).reshape(
                2, 128, D).transpose(1, 0, 2))
        in_maps.append({
            "x8": xt.astype(F8),
            "xb": xt.astype(BF),
            "wq8": wq8.astype(F8),
            "wk8": wk8.astype(F8),
            "wvb": wvb.astype(BF),
            "wob": wob.astype(BF),
            "cosb": cos2.astype(BF),
            "sinb": sins.astype(BF),
            "mskb": mvals.astype(BF),
            "conf": consts,
        })
    _LAST_INMAPS = in_maps

    res = run_bass_kernel_spmd(_get_nc(), in_maps,
                               core_ids=list(range(NCORES)))
    out = np.zeros((B, T, D), dtype=np.float32)
    for b in range(B):
        acc = np.zeros((T, D), dtype=np.float32)
        for g in range(4):
            acc += np.asarray(res.results[4 * b + g]["yp"],
                              dtype=np.float32)
        out[b] = acc
    return out


def bench(trace=True):
    """Re-run last inputs with NTFF tracing; returns BassKernelResults."""
    assert _LAST_INMAPS is not None, "call kernel() first"
    return run_bass_kernel_spmd(_get_nc(), _LAST_INMAPS,
                                core_ids=list(range(NCORES)), trace=trace)


# revision 54
# speedup vs baseline: 1.4684x; 1.1484x over previous
"""BinarySelfAttention Trainium2 kernel (8-core SPMD).

Strategy: shard (batch, head-group): core c -> batch c//4, heads 4*(c%4)..+3.
Each core computes ternary-projected QKV for its 4 heads, RoPE, causal
flash-style attention in S^T orientation (keys on partitions -> no transposes),
and a partial output projection against its Wo column slice. Host sums the 4
partials per batch.

Precision plan (cost model: bf16 matmul = 1 cycle/row at any width; fp8e4 +
DoubleRow = 0.5 cycles/row; f32r pays 4x on <256-wide chunks):
- Q/K projections: fp8e4 x and ternary signs, DoubleRow perf mode (256-deep
  contraction per instruction). Scores only shift ~1% from fp8 x.
- Everything else (V proj, S, PV, out proj): bf16 operands, f32 PSUM.
- Ternary scales fold into the exp() scale (sq*sk/8, runtime data) and into
  the host-prepared Wo slice (sv*so), keeping the program input-independent.

Schedule: PE stream is software-pipelined (S piece kc+1 issued before PV of
kc so exp latency hides behind matmuls); rope of head-pair 1 is deferred into
head 0/1's attention window; reciprocal broadcasts bounce through DRAM in
head pairs (one write + one read per pair).
"""
import numpy as np

import concourse.bass as bass
import concourse.mybir as mybir
import concourse.tile as tile
from concourse.bass_utils import run_bass_kernel_spmd
from concourse.tile_rust import add_dep_helper

F32 = mybir.dt.float32
BF16 = mybir.dt.bfloat16
FP8 = mybir.dt.float8e4
DR = mybir.MatmulPerfMode.DoubleRow

B, T, D, H = 2, 2048, 1024, 16
HD = 64            # head dim
HPC = 4            # heads per core
FPC = HPC * HD     # features per core (256)
NCORES = 8
KC = D // 128      # 8 contraction chunks of 128


def _split_excess_waits(nc, max_waits=1):
    """TRN2 ISA has one sem-wait slot per instruction and this walrus build
    rejects 3+; hoist excess waits onto preceding same-engine NOPs."""
    n = 0
    for f in nc.m.functions:
        for bb in f.blocks:
            new_insts = []
            for inst in bb.instructions:
                si = getattr(inst, 'sync_info', None)
                if si is not None and si.on_wait and len(si.on_wait) > max_waits:
                    waits = list(si.on_wait)
                    extra, keep = waits[:-max_waits], waits[-max_waits:]
                    for j, w in enumerate(extra):
                        new_insts.append(mybir.InstNoOp(
                            name=f"{inst.name}-wsplit{j}",
                            engine=inst.engine,
                            sync_info=mybir.SyncInfo(on_wait=[w], on_update=[]),
                            bass_nofuse=True,
                        ))
                        n += 1
                    inst.sync_info = mybir.SyncInfo(
                        on_wait=keep, on_update=si.on_update)
                new_insts.append(inst)
            bb.instructions[:] = new_insts
    return n


def _build():
    nc = bass.Bass("TRN2", target_bir_lowering=False, debug=False,
                   num_devices=NCORES)
    x8_d = nc.dram_tensor("x8", [D, T], FP8, kind="ExternalInput")
    xb_d = nc.dram_tensor("xb", [D, T], BF16, kind="ExternalInput")
    wq_d = nc.dram_tensor("wq8", [128, 8 * FPC], FP8, kind="ExternalInput")
    wk_d = nc.dram_tensor("wk8", [128, 8 * FPC], FP8, kind="ExternalInput")
    wv_d = nc.dram_tensor("wvb", [128, KC, FPC], BF16, kind="ExternalInput")
    wo_d = nc.dram_tensor("wob", [128, 2, D], BF16, kind="ExternalInput")
    cos_d = nc.dram_tensor("cosb", [128, T], BF16, kind="ExternalInput")
    sin_d = nc.dram_tensor("sinb", [128, T], BF16, kind="ExternalInput")
    msk_d = nc.dram_tensor("mskb", [128, 128], BF16, kind="ExternalInput")
    con_d = nc.dram_tensor("conf", [128, 1], F32, kind="ExternalInput")
    yp_d = nc.dram_tensor("yp", [T, D], BF16, kind="ExternalOutput")
    rec_d = nc.dram_tensor("recd", [2, 2, T], BF16)  # internal scratch

    EXP = mybir.ActivationFunctionType.Exp

    with tile.TileContext(nc) as tc:
        with tc.tile_pool(name="main", bufs=1) as mp:
            X8 = mp.tile([128, KC, T], FP8)
            XB = mp.tile([128, KC, T], BF16)
            W8Q = mp.tile([128, 4, 2, FPC], FP8)
            W8K = mp.tile([128, 4, 2, FPC], FP8)
            WV = mp.tile([128, KC, FPC], BF16)
            WOC = mp.tile([128, 2, D], BF16)
            COS = mp.tile([128, T], BF16)
            SIN = mp.tile([128, T], BF16)
            MSK = mp.tile([128, 128], BF16)
            CON = mp.tile([128, 1], F32)
            QT = [mp.tile([128, T], BF16, tag=f"qt{i}", name=f"qt{i}")
                  for i in range(2)]
            KT = [mp.tile([128, T], BF16, tag=f"kt{i}", name=f"kt{i}")
                  for i in range(2)]
            VA = mp.tile([128, 16, HPC * 65], BF16)
            AT = [mp.tile([128, T], BF16, tag=f"at{i}", name=f"at{i}")
                  for i in range(2)]

            # ---------------- DMA preamble ----------------
            # scalar queue: weights + tables; sync queue: activations.
            # (rot-swap DMAs go on the DVE queue so nothing blocks them.)
            nc.scalar.dma_start(out=W8K.rearrange("p a b f -> p (a b f)"),
                                in_=wk_d[:, :])
            nc.scalar.dma_start(out=W8Q.rearrange("p a b f -> p (a b f)"),
                                in_=wq_d[:, :])
            for kp in range(4):  # x8 in kc pairs (pair 0 split for warmup)
                for (t0, t1) in ([(0, 1024), (1024, T)] if kp == 0
                                 else [(0, T)]):
                    src = x8_d[256 * kp:256 * kp + 256, t0:t1]
                    nc.sync.dma_start(
                        out=X8[:, 2 * kp:2 * kp + 2, t0:t1],
                        in_=bass.AP(tensor=src.tensor, offset=src.offset,
                                    ap=[[T, 128], [128 * T, 2],
                                        [1, t1 - t0]]))
            nc.sync.dma_start(out=COS[:, 0:1024], in_=cos_d[:, 0:1024])
            nc.sync.dma_start(out=SIN[:, 0:1024], in_=sin_d[:, 0:1024])
            nc.sync.dma_start(out=WV, in_=wv_d[:, :, :])
            # xb in fine-grained T-blocks: V projection streams early and
            # the rot-swap DMAs (sync queue) never wait long for the DMA
            # engines behind a bulk transfer.
            nc.sync.dma_start(out=MSK, in_=msk_d[:, :])
            nc.sync.dma_start(out=CON, in_=con_d[:, :])
            for tb in range(8):
                src = xb_d[0:128, 256 * tb:256 * tb + 256]
                q = nc.sync
                q.dma_start(
                    out=XB[:, :, 256 * tb:256 * tb + 256],
                    in_=bass.AP(tensor=src.tensor, offset=src.offset,
                                ap=[[T, 128], [128 * T, KC], [1, 256]]))
            nc.sync.dma_start(out=COS[:, 1024:T], in_=cos_d[:, 1024:T])
            nc.sync.dma_start(out=SIN[:, 1024:T], in_=sin_d[:, 1024:T])
            nc.scalar.dma_start(out=WOC, in_=wo_d[:, :, :])

            ones_view = VA[:, :, :].rearrange(
                "p a (h e) -> p a h e", e=65)[:, :, :, 64:65].rearrange(
                "p a h e -> p (a h e)")
            nc.vector.memset(ones_view, 1.0)

            # ---------------- Phase 1: QK projections (fp8 DoubleRow) ------
            psqk_cm = tc.tile_pool(name="psqk", bufs=2, space="PSUM")
            psqk = psqk_cm.__enter__()

            def proj_qk(wt, dst, nm, fh):
                accs = [psqk.tile([128, 512], F32, tag=f"pa{t}",
                                  name=f"{nm}{fh}a{t}")
                        for t in range(4)]
                for kcp in range(4):
                    for tch in range(4):
                        nc.tensor.matmul(
                            accs[tch],
                            wt[:, kcp, :, 128 * fh:128 * fh + 128],
                            X8[:, 2 * kcp:2 * kcp + 2,
                               512 * tch:512 * tch + 512],
                            start=(kcp == 0), stop=(kcp == 3),
                            perf_mode=DR)
                for tch in range(4):
                    # fh0 evicts on DVE (feed rope asap); fh1 on the
                    # Activation engine, idle before the exp stream starts
                    eng = (nc.vector.tensor_copy if fh == 0
                           else nc.scalar.copy)
                    eng(out=dst[fh][:, 512 * tch:512 * tch + 512],
                        in_=accs[tch])

            # ---------------- RoPE (bf16, DVE; rot swap via DMA) ----------
            rp_cm = tc.tile_pool(name="rp", bufs=2)
            rp = rp_cm.__enter__()

            def rope(dst, pfx, c0=0, c1=T, dq=None):
                rot = rp.tile([128, T], BF16, tag="rot", name=f"{pfx}rot")
                for g in range(4):
                    b0 = 32 * g
                    s0 = 32 * (g ^ 1)
                    (dq or nc.sync).dma_start(out=rot[b0:b0 + 32, c0:c1],
                                              in_=dst[s0:s0 + 32, c0:c1])
                nc.vector.tensor_mul(rot[:, c0:c1], rot[:, c0:c1],
                                     SIN[:, c0:c1])
                nc.vector.tensor_mul(dst[:, c0:c1], dst[:, c0:c1],
                                     COS[:, c0:c1])
                nc.vector.tensor_add(dst[:, c0:c1], dst[:, c0:c1],
                                     rot[:, c0:c1])

            # head-pair 0 / first column half races through projection,
            # eviction, and rope so the exp stream starts early.
            proj_qk(W8K, KT, "k", 0)
            proj_qk(W8Q, QT, "q", 0)
            rope(KT[0], "k0", 0, 1024, dq=nc.scalar)
            rope(QT[0], "q0", 0, 1024, dq=nc.scalar)
            proj_qk(W8Q, QT, "q", 1)
            proj_qk(W8K, KT, "k", 1)
            psqk_cm.__exit__(None, None, None)

            # ---- Phase 1c/2: V projection + attention (pipelined) -------
            pss_cm = tc.tile_pool(name="pss", bufs=2, space="PSUM")
            pss = pss_cm.__enter__()
            psv_cm = tc.tile_pool(name="psv", bufs=2, space="PSUM")
            psv = psv_cm.__enter__()
            ptp_cm = tc.tile_pool(name="pt", bufs=20)
            ptp = ptp_cm.__enter__()

            def vproj(t16):
                acc = psv.tile([128, FPC], F32, tag="pv")
                for kc in range(KC):
                    nc.tensor.matmul(
                        acc,
                        XB[:, kc, 128 * t16:128 * t16 + 128],
                        WV[:, kc, :],
                        start=(kc == 0), stop=(kc == KC - 1))
                eng = nc.vector.tensor_copy
                eng(out=VA[:, t16, :].rearrange(
                        "p (h e) -> p h e", e=65)[:, :, 0:64],
                    in_=acc.rearrange("p (h e) -> p h e", e=64))

            _mask_eng = [None]  # None = alternate
            _alt = [0]

            def s_exp_piece(h, qh, kc, pool, ptag):
                qt, kt = QT[h // 2], KT[h // 2]
                r0 = 64 * (h % 2)
                q0, q1 = 1024 * qh, 1024 * qh + 1024
                qs = max(q0, 128 * kc)
                cols = q1 - qs
                sp = pss.tile([128, 1024], F32, tag="sp")
                off = 0
                while off < cols:
                    # matmul must not cross a 512-f32 PSUM bank edge
                    cw = min(512 - (off % 512), cols - off)
                    nc.tensor.matmul(
                        sp[:, off:off + cw],
                        kt[r0:r0 + 64, 128 * kc:128 * kc + 128],
                        qt[r0:r0 + 64, qs + off:qs + off + cw],
                        start=True, stop=True)
                    off += cw
                pt = pool.tile([128, 1024], BF16, tag=ptag)
                nc.scalar.activation(
                    out=pt[:, 0:cols], in_=sp[:, 0:cols],
                    func=EXP, scale=CON[:, 0:1])
                if qs == 128 * kc:  # diagonal block leads piece
                    if _mask_eng[0] is not None:
                        eng = _mask_eng[0]
                    else:
                        _alt[0] += 1
                        eng = (nc.vector.tensor_mul if _alt[0] % 2 == 0
                               else nc.gpsimd.tensor_mul)
                    eng(pt[:, 0:128], pt[:, 0:128], MSK)
                return pt, qs, cols

            def pv_piece(yaug, h, qh, kc, pt, qs, cols):
                # The diagonal (masked) 0:128 chunk is emitted LAST so the
                # unmasked bulk of PV never waits on the mask multiply.
                q0 = 1024 * qh
                diag = (qs == 128 * kc and cols > 128)
                off = 128 if diag else 0
                chunks = []
                while off < cols:
                    cw = min(512 - ((qs + off) % 512), cols - off)
                    chunks.append((off, cw))
                    off += cw
                if diag:
                    chunks.append((0, 128))
                started = set()
                for off, cw in chunks:
                    w = (qs + off) // 512
                    st = (kc == 0) and (w not in started)
                    if kc == 0:
                        started.add(w)
                    nc.tensor.matmul(
                        yaug[:, qs - q0 + off:qs - q0 + off + cw],
                        VA[:, kc, 65 * h:65 * h + 65],
                        pt[:, off:off + cw],
                        start=st, stop=(kc == 4 * w + 3))

            # ------- Phase 2: decoupled S/exp stream + lagged PV stream ----
            # The exp stream (Activation engine) is the global bottleneck:
            # S+exp pieces are emitted in one continuous stream (keeping the
            # scalar engine fed), while the PV/normalization consumer runs
            # LAG pieces behind, and the V projection weaves into the early
            # stream. pt tiles buffer the in-flight pieces.
            psy_cm = tc.tile_pool(name="psy", bufs=2, space="PSUM")
            p2_cm = tc.tile_pool(name="p2", bufs=2)
            p2 = p2_cm.__enter__()
            p4_cm = tc.tile_pool(name="p4", bufs=4)
            p4 = p4_cm.__enter__()
            rb_cm = tc.tile_pool(name="rb", bufs=2)
            rbp = rb_cm.__enter__()

            norm_state = {}
            LAG = 16

            def normalize(h, qh, yaug, last_pair):
                """Softmax denominator: stage, reciprocal, DRAM-bounce
                broadcast per head pair, then the normalizing muls."""
                q0, q1 = 1024 * qh, 1024 * qh + 1024
                hp = h // 2
                # Single stage-copy (65 rows incl. denominator) releases
                # the yaug PSUM buffer early. qh1 odd heads skip it (their
                # long period hides the bounce; mul reads PSUM directly).
                if qh == 0 or h % 2 == 0:
                    ystg = p4.tile([65, 1024], BF16, tag="ystg")
                    nc.vector.tensor_copy(out=ystg, in_=yaug)
                    den_src, y_src, y_psum = (ystg[64:65, :],
                                              ystg[0:64, :], 0)
                else:
                    den_src, y_src, y_psum = (yaug[64:65, :],
                                              yaug[0:64, :], 1)
                if h % 2 == 0:
                    rec2 = p4.tile([1, 2, 1024], BF16, tag="rec")
                    with nc.allow_low_precision(reason="rec bounce"):
                        nc.vector.reciprocal(out=rec2[:, 0, :], in_=den_src)
                    norm_state[(hp, qh)] = (rec2, y_src)
                    return
                rec2, ysrc_e = norm_state.pop((hp, qh))
                with nc.allow_low_precision(reason="rec bounce"):
                    nc.vector.reciprocal(out=rec2[:, 1, :], in_=den_src)
                wr_i = nc.sync.dma_start(
                    out=rec_d[hp, :, q0:q1],
                    in_=rec2.rearrange("p a c -> p (a c)"))
                rb2 = rbp.tile([64, 2, 1024], BF16, tag="rb")
                rsrc = rec_d[hp, 0, q0:q1]
                rd_i = nc.sync.dma_start(
                    out=rb2,
                    in_=bass.AP(tensor=rsrc.tensor, offset=rsrc.offset,
                                ap=[[0, 64], [T, 2]] + list(rsrc.ap)))
                # Tile does not track DRAM scratch RAW deps
                add_dep_helper(rd_i.ins, wr_i.ins, sync=True,
                               reason="recd bounce RAW")
                # even head -> AT rows 0:64 directly; odd staged + DMA.
                # Off the critical path these go to the Pool engine; the
                # final pair stays on DVE (2x bf16) to shorten the tail.
                meng = (nc.vector.tensor_mul if (last_pair or y_psum)
                        else nc.gpsimd.tensor_mul)
                meng2 = (nc.vector.tensor_mul if last_pair
                         else nc.gpsimd.tensor_mul)
                meng2(AT[hp][0:64, q0:q1], ysrc_e, rb2[:, 0, :])
                stg = p2.tile([64, 1024], BF16, tag="stg")
                meng(stg, y_src, rb2[:, 1, :])
                nc.sync.dma_start(out=AT[hp][64:128, q0:q1], in_=stg)

            head_order = [(0, 0), (1, 0), (2, 0), (3, 0),
                          (2, 1), (3, 1), (0, 1), (1, 1)]
            plist = [(h, qh, kc) for (h, qh) in head_order
                     for kc in range(8 * (qh + 1))]
            pieces = {}
            cons = {"i": 0, "yaug": None}

            def consume_one():
                h, qh, kc = plist[cons["i"]]
                cons["i"] += 1
                if kc == 0:
                    cons["yaug"] = psy.tile([65, 1024], F32, tag="yaug",
                                            name=f"ya{h}_{qh}")
                pv_piece(cons["yaug"], h, qh, kc, *pieces.pop((h, qh, kc)))
                if kc == 8 * (qh + 1) - 1:
                    normalize(h, qh, cons["yaug"], (h, qh) in
                              ((0, 1), (1, 1)))

            _mask_eng[0] = nc.gpsimd.tensor_mul
            vproj(0)
            vproj(1)
            psy = None
            for i, (h, qh, kc) in enumerate(plist):
                pieces[(h, qh, kc)] = s_exp_piece(h, qh, kc, ptp, "pt")
                if 2 <= i <= 15:
                    vproj(i)
                if i == 15:
                    psv_cm.__exit__(None, None, None)
                    psy = psy_cm.__enter__()
                if i == 9:
                    rope(QT[1], "q1", 0, 1024)
                    rope(KT[1], "k1", 0, 1024)
                if i == 24:
                    rope(QT[0], "q0b", 1024, T)
                    rope(KT[0], "k0b", 1024, T)
                    rope(QT[1], "q1b", 1024, T)
                    rope(KT[1], "k1b", 1024, T)
                if i == 63:
                    _mask_eng[0] = None
                if i >= LAG:
                    consume_one()

            # ---------------- Phase 3: output projection ----------------
            # Reuses the attention pools (sp tiles for PSUM accumulation,
            # pt tiles for the bf16 eviction staging) so no pool barrier
            # separates it from the final normalization.
            def oproj(t16):
                yo = pss.tile([128, D], F32, tag="sp", name=f"yo{t16}")
                for half in range(2):
                    for ft in (1, 0):
                        nc.tensor.matmul(
                            yo[:, 512 * half:512 * half + 512],
                            AT[ft][:, 128 * t16:128 * t16 + 128],
                            WOC[:, ft, 512 * half:512 * half + 512],
                            start=(ft == 1), stop=(ft == 0))
                return yo

            def oevict(t16, yo):
                ot = ptp.tile([128, D], BF16, tag="pt", name=f"ot{t16}")
                eng = (nc.scalar.copy, nc.vector.tensor_copy)[t16 % 2]
                eng(out=ot, in_=yo)
                deng = nc.sync if t16 % 2 == 0 else nc.scalar
                deng.dma_start(
                    out=yp_d[128 * t16:128 * t16 + 128, :], in_=ot)

            prev = None
            t16 = 0
            while cons["i"] < len(plist):
                consume_one()
                if cons["i"] % 2 == 0 and t16 < 8:
                    yo = oproj(t16)
                    if prev is not None:
                        oevict(t16 - 1, prev)
                    prev = yo
                    t16 += 1
            while t16 < 16:
                yo = oproj(t16)
                if prev is not None:
                    oevict(t16 - 1, prev)
                prev = yo
                t16 += 1
            oevict(15, prev)

            psy_cm.__exit__(None, None, None)
            pss_cm.__exit__(None, None, None)
            rb_cm.__exit__(None, None, None)
            p4_cm.__exit__(None, None, None)
            p2_cm.__exit__(None, None, None)
            ptp_cm.__exit__(None, None, None)
            rp_cm.__exit__(None, None, None)

    _split_excess_waits(nc)
    return nc


_NC = None
_LAST_INMAPS = None


def _get_nc():
    global _NC
    if _NC is None:
        _NC = _build()
    return _NC


def _ternary_signs(w):
    """Mirror reference ternary_weight: returns (signs in {-1,0,1}, scale)."""
    try:
        import jax
        import jax.numpy as jnp
        cpu = jax.devices("cpu")[0]
        with jax.default_device(cpu):
            wj = jnp.asarray(np.asarray(w, dtype=np.float32))
            scale = jnp.mean(jnp.abs(wj))
            signs = jnp.round(jnp.clip(wj / (scale + 1e-8), -1.0, 1.0))
            return np.asarray(signs, dtype=np.float32), float(scale)
    except Exception:
        w = np.asarray(w, dtype=np.float32)
        scale = np.float32(np.mean(np.abs(w)))
        signs = np.round(np.clip(w / (scale + np.float32(1e-8)), -1.0, 1.0))
        return signs.astype(np.float32), float(scale)


def _rope_tables():
    inv = (1.0 / (10000.0 ** (np.arange(0, HD, 2, dtype=np.float32) / HD))
           ).astype(np.float32)                      # [32]
    t = np.arange(T, dtype=np.float32)
    fr = np.outer(t, inv).astype(np.float32)         # [T, 32]
    cos1 = np.cos(fr).astype(np.float32)             # [T, 32]
    sin1 = np.sin(fr).astype(np.float32)
    # rows: d in 0..63 (freq d%32), tiled for 2 heads -> 128 rows
    cosd = np.concatenate([cos1, cos1], axis=1).T    # [64, T]
    sind = np.concatenate([sin1, sin1], axis=1).T    # [64, T]
    sgn = np.ones((HD, 1), dtype=np.float32)
    sgn[:HD // 2] = -1.0
    cos2 = np.tile(cosd, (2, 1)).astype(np.float32)          # [128, T]
    sins = np.tile(sind * sgn, (2, 1)).astype(np.float32)    # [128, T]
    return cos2, sins


def kernel(x, Wq, Wk, Wv, Wo, mask):
    global _LAST_INMAPS
    import ml_dtypes
    F8 = ml_dtypes.float8_e4m3
    BF = ml_dtypes.bfloat16

    x = np.asarray(x, dtype=np.float32)
    mask = np.asarray(mask)
    assert np.array_equal(
        np.asarray(mask[0, 0], dtype=np.int32),
        np.tril(np.ones((T, T), dtype=np.int32))), "non-causal mask"

    qs, sq = _ternary_signs(Wq)
    ks, sk = _ternary_signs(Wk)
    vs, sv = _ternary_signs(Wv)
    os_, so = _ternary_signs(Wo)
    cos2, sins = _rope_tables()
    mvals = np.triu(np.ones((128, 128), dtype=np.float32))  # valid: k <= q
    consts = np.full((128, 1), np.float32(sq) * np.float32(sk) *
                     np.float32(0.125), dtype=np.float32)

    in_maps = []
    for c in range(NCORES):
        b, g = c // 4, c % 4
        fsl = slice(FPC * g, FPC * g + FPC)
        xt = np.ascontiguousarray(x[b].T)            # [D, T]
        # DR weight layout: w8[p, kcp, i, f] = signs[FPC*g+f, 256*kcp+128*i+p]
        wq8 = np.ascontiguousarray(
            qs[fsl].T.reshape(4, 2, 128, FPC).transpose(2, 0, 1, 3)
            ).reshape(128, 8 * FPC)
        wk8 = np.ascontiguousarray(
            ks[fsl].T.reshape(4, 2, 128, FPC).transpose(2, 0, 1, 3)
            ).reshape(128, 8 * FPC)
        wvb = np.ascontiguousarray(
            vs[fsl].T.reshape(KC, 128, FPC).transpose(1, 0, 2))
        wob = np.ascontiguousarray(
            (os_[:, fsl].T * np.float32(sv * so)).reshape(
                2, 128, D).transpose(1, 0, 2))
        in_maps.append({
            "x8": xt.astype(F8),
            "xb": xt.astype(BF),
            "wq8": wq8.astype(F8),
            "wk8": wk8.astype(F8),
            "wvb": wvb.astype(BF),
            "wob": wob.astype(BF),
            "cosb": cos2.astype(BF),
            "sinb": sins.astype(BF),
            "mskb": mvals.astype(BF),
            "conf": consts,
        })
    _LAST_INMAPS = in_maps

    res = run_bass_kernel_spmd(_get_nc(), in_maps,
                               core_ids=list(range(NCORES)))
    out = np.zeros((B, T, D), dtype=np.float32)
    for b in range(B):
        acc = np.zeros((T, D), dtype=np.float32)
        for g in range(4):
            acc += np.asarray(res.results[4 * b + g]["yp"],
                              dtype=np.float32)
        out[b] = acc
    return out


def bench(trace=True):
    """Re-run last inputs with NTFF tracing; returns BassKernelResults."""
    assert _LAST_INMAPS is not None, "call kernel() first"
    return run_bass_kernel_spmd(_get_nc(), _LAST_INMAPS,
                                core_ids=list(range(NCORES)), trace=trace)


# revision 77
# speedup vs baseline: 1.5304x; 1.0423x over previous
"""BinarySelfAttention Trainium2 kernel (8-core SPMD).

Strategy: shard (batch, head-group): core c -> batch c//4, heads 4*(c%4)..+3.
Each core computes ternary-projected QKV for its 4 heads, RoPE, causal
flash-style attention in S^T orientation (keys on partitions -> no transposes),
and a partial output projection against its Wo column slice. Host sums the 4
partials per batch.

Precision plan (cost model: bf16 matmul = 1 cycle/row at any width; fp8e4 +
DoubleRow = 0.5 cycles/row; f32r pays 4x on <256-wide chunks):
- Q/K projections: fp8e4 x and ternary signs, DoubleRow perf mode (256-deep
  contraction per instruction). Scores only shift ~1% from fp8 x.
- Everything else (V proj, S, PV, out proj): bf16 operands, f32 PSUM.
- Ternary scales fold into the exp() scale (sq*sk/8, runtime data) and into
  the host-prepared Wo slice (sv*so), keeping the program input-independent.

Schedule: PE stream is software-pipelined (S piece kc+1 issued before PV of
kc so exp latency hides behind matmuls); rope of head-pair 1 is deferred into
head 0/1's attention window; reciprocal broadcasts bounce through DRAM in
head pairs (one write + one read per pair).
"""
import numpy as np

import concourse.bass as bass
import concourse.mybir as mybir
import concourse.tile as tile
from concourse.bass_utils import run_bass_kernel_spmd
from concourse.tile_rust import add_dep_helper

F32 = mybir.dt.float32
BF16 = mybir.dt.bfloat16
FP8 = mybir.dt.float8e4
DR = mybir.MatmulPerfMode.DoubleRow

B, T, D, H = 2, 2048, 1024, 16
HD = 64            # head dim
HPC = 4            # heads per core
FPC = HPC * HD     # features per core (256)
NCORES = 8
KC = D // 128      # 8 contraction chunks of 128


def _split_excess_waits(nc, max_waits=1):
    """TRN2 ISA has one sem-wait slot per instruction and this walrus build
    rejects 3+; hoist excess waits onto preceding same-engine NOPs."""
    n = 0
    for f in nc.m.functions:
        for bb in f.blocks:
            new_insts = []
            for inst in bb.instructions:
                si = getattr(inst, 'sync_info', None)
                if si is not None and si.on_wait and len(si.on_wait) > max_waits:
                    waits = list(si.on_wait)
                    extra, keep = waits[:-max_waits], waits[-max_waits:]
                    for j, w in enumerate(extra):
                        new_insts.append(mybir.InstNoOp(
                            name=f"{inst.name}-wsplit{j}",
                            engine=inst.engine,
                            sync_info=mybir.SyncInfo(on_wait=[w], on_update=[]),
                            bass_nofuse=True,
                        ))
                        n += 1
                    inst.sync_info = mybir.SyncInfo(
                        on_wait=keep, on_update=si.on_update)
                new_insts.append(inst)
            bb.instructions[:] = new_insts
    return n


def _build():
    nc = bass.Bass("TRN2", target_bir_lowering=False, debug=False,
                   num_devices=NCORES)
    x8_d = nc.dram_tensor("x8", [D, T], FP8, kind="ExternalInput")
    xb_d = nc.dram_tensor("xb", [D, T], BF16, kind="ExternalInput")
    wq_d = nc.dram_tensor("wq8", [128, 8 * FPC], FP8, kind="ExternalInput")
    wk_d = nc.dram_tensor("wk8", [128, 8 * FPC], FP8, kind="ExternalInput")
    wv_d = nc.dram_tensor("wvb", [128, KC, FPC], BF16, kind="ExternalInput")
    wo_d = nc.dram_tensor("wob", [128, 2, D], BF16, kind="ExternalInput")
    cos_d = nc.dram_tensor("cosb", [128, T], BF16, kind="ExternalInput")
    sin_d = nc.dram_tensor("sinb", [128, T], BF16, kind="ExternalInput")
    msk_d = nc.dram_tensor("mskb", [128, 128], BF16, kind="ExternalInput")
    con_d = nc.dram_tensor("conf", [128, 1], F32, kind="ExternalInput")
    yp_d = nc.dram_tensor("yp", [T, D], BF16, kind="ExternalOutput")
    rec_d = nc.dram_tensor("recd", [2, 2, T], BF16)  # internal scratch

    EXP = mybir.ActivationFunctionType.Exp

    with tile.TileContext(nc) as tc:
        with tc.tile_pool(name="main", bufs=1) as mp:
            X8 = mp.tile([128, KC, T], FP8)
            XB = mp.tile([128, KC, T], BF16)
            W8Q = mp.tile([128, 4, 2, FPC], FP8)
            W8K = mp.tile([128, 4, 2, FPC], FP8)
            WV = mp.tile([128, KC, FPC], BF16)
            WOC = mp.tile([128, 2, D], BF16)
            COS = mp.tile([128, T], BF16)
            SIN = mp.tile([128, T], BF16)
            MSK = mp.tile([128, 128], BF16)
            CON = mp.tile([128, 1], F32)
            QT = [mp.tile([128, T], BF16, tag=f"qt{i}", name=f"qt{i}")
                  for i in range(2)]
            KT = [mp.tile([128, T], BF16, tag=f"kt{i}", name=f"kt{i}")
                  for i in range(2)]
            VA = mp.tile([128, 16, HPC * 65], BF16)
            AT = [mp.tile([128, T], BF16, tag=f"at{i}", name=f"at{i}")
                  for i in range(2)]

            # ---------------- DMA preamble ----------------
            # scalar queue: weights + tables; sync queue: activations.
            # (rot-swap DMAs go on the DVE queue so nothing blocks them.)
            nc.scalar.dma_start(out=W8K.rearrange("p a b f -> p (a b f)"),
                                in_=wk_d[:, :])
            nc.scalar.dma_start(out=W8Q.rearrange("p a b f -> p (a b f)"),
                                in_=wq_d[:, :])
            for kp in range(4):  # x8 in kc pairs (pair 0 split for warmup)
                for (t0, t1) in ([(0, 1024), (1024, T)] if kp == 0
                                 else [(0, T)]):
                    src = x8_d[256 * kp:256 * kp + 256, t0:t1]
                    nc.sync.dma_start(
                        out=X8[:, 2 * kp:2 * kp + 2, t0:t1],
                        in_=bass.AP(tensor=src.tensor, offset=src.offset,
                                    ap=[[T, 128], [128 * T, 2],
                                        [1, t1 - t0]]))
            nc.sync.dma_start(out=COS, in_=cos_d[:, :])
            nc.sync.dma_start(out=SIN, in_=sin_d[:, :])
            nc.sync.dma_start(out=WV, in_=wv_d[:, :, :])
            # xb in fine-grained T-blocks: V projection streams early and
            # the rot-swap DMAs (sync queue) never wait long for the DMA
            # engines behind a bulk transfer.
            nc.sync.dma_start(out=MSK, in_=msk_d[:, :])
            nc.sync.dma_start(out=CON, in_=con_d[:, :])
            def xb_load(tb):
                src = xb_d[0:128, 256 * tb:256 * tb + 256]
                nc.sync.dma_start(
                    out=XB[:, :, 256 * tb:256 * tb + 256],
                    in_=bass.AP(tensor=src.tensor, offset=src.offset,
                                ap=[[T, 128], [128 * T, KC], [1, 256]]))

            for tb in range(4):
                xb_load(tb)
            nc.scalar.dma_start(out=WOC, in_=wo_d[:, :, :])

            ones_view = VA[:, :, :].rearrange(
                "p a (h e) -> p a h e", e=65)[:, :, :, 64:65].rearrange(
                "p a h e -> p (a h e)")
            nc.vector.memset(ones_view, 1.0)

            # ---------------- Phase 1: QK projections (fp8 DoubleRow) ------
            psqk_cm = tc.tile_pool(name="psqk", bufs=2, space="PSUM")
            psqk = psqk_cm.__enter__()

            def proj_qk(wt, dst, nm, fh):
                accs = [psqk.tile([128, 512], F32, tag=f"pa{t}",
                                  name=f"{nm}{fh}a{t}")
                        for t in range(4)]
                for kcp in range(4):
                    for tch in range(4):
                        nc.tensor.matmul(
                            accs[tch],
                            wt[:, kcp, :, 128 * fh:128 * fh + 128],
                            X8[:, 2 * kcp:2 * kcp + 2,
                               512 * tch:512 * tch + 512],
                            start=(kcp == 0), stop=(kcp == 3),
                            perf_mode=DR)
                for tch in range(4):
                    # fh0 evicts on DVE (feed rope asap); fh1 on the
                    # Activation engine, idle before the exp stream starts
                    eng = (nc.vector.tensor_copy if fh == 0
                           else nc.scalar.copy)
                    eng(out=dst[fh][:, 512 * tch:512 * tch + 512],
                        in_=accs[tch])

            # ---------------- RoPE (bf16, DVE; rot swap via DMA) ----------
            rp_cm = tc.tile_pool(name="rp", bufs=2)
            rp = rp_cm.__enter__()

            def rope(dst, pfx, c0=0, c1=T, dq=None):
                rot = rp.tile([128, T], BF16, tag="rot", name=f"{pfx}rot")
                for g in range(4):
                    b0 = 32 * g
                    s0 = 32 * (g ^ 1)
                    (dq or nc.sync).dma_start(out=rot[b0:b0 + 32, c0:c1],
                                              in_=dst[s0:s0 + 32, c0:c1])
                nc.vector.tensor_mul(rot[:, c0:c1], rot[:, c0:c1],
                                     SIN[:, c0:c1])
                nc.vector.tensor_mul(dst[:, c0:c1], dst[:, c0:c1],
                                     COS[:, c0:c1])
                nc.vector.tensor_add(dst[:, c0:c1], dst[:, c0:c1],
                                     rot[:, c0:c1])

            # head-pair 0 / first column half races through projection,
            # eviction, and rope so the exp stream starts early.
            proj_qk(W8K, KT, "k", 0)
            proj_qk(W8Q, QT, "q", 0)
            rope(KT[0], "k0", 0, 1024)
            rope(QT[0], "q0", 0, 1024)
            proj_qk(W8Q, QT, "q", 1)
            proj_qk(W8K, KT, "k", 1)
            psqk_cm.__exit__(None, None, None)

            # ---- Phase 1c/2: V projection + attention (pipelined) -------
            pss_cm = tc.tile_pool(name="pss", bufs=3, space="PSUM")
            pss = pss_cm.__enter__()
            psv_cm = tc.tile_pool(name="psv", bufs=2, space="PSUM")
            psv = psv_cm.__enter__()
            ptp_cm = tc.tile_pool(name="pt", bufs=20)
            ptp = ptp_cm.__enter__()

            def vproj(t16):
                acc = psv.tile([128, FPC], F32, tag="pv")
                for kc in range(KC):
                    nc.tensor.matmul(
                        acc,
                        XB[:, kc, 128 * t16:128 * t16 + 128],
                        WV[:, kc, :],
                        start=(kc == 0), stop=(kc == KC - 1))
                eng = nc.vector.tensor_copy
                eng(out=VA[:, t16, :].rearrange(
                        "p (h e) -> p h e", e=65)[:, :, 0:64],
                    in_=acc.rearrange("p (h e) -> p h e", e=64))

            _mask_eng = [None]  # None = alternate
            _alt = [0]

            def s_exp_piece(h, qh, kc, pool, ptag):
                qt, kt = QT[h // 2], KT[h // 2]
                r0 = 64 * (h % 2)
                q0, q1 = 1024 * qh, 1024 * qh + 1024
                qs = max(q0, 128 * kc)
                cols = q1 - qs
                sp = pss.tile([128, 1024], F32, tag="sp")
                off = 0
                while off < cols:
                    # matmul must not cross a 512-f32 PSUM bank edge
                    cw = min(512 - (off % 512), cols - off)
                    nc.tensor.matmul(
                        sp[:, off:off + cw],
                        kt[r0:r0 + 64, 128 * kc:128 * kc + 128],
                        qt[r0:r0 + 64, qs + off:qs + off + cw],
                        start=True, stop=True)
                    off += cw
                pt = pool.tile([128, 1024], BF16, tag=ptag)
                nc.scalar.activation(
                    out=pt[:, 0:cols], in_=sp[:, 0:cols],
                    func=EXP, scale=CON[:, 0:1])
                if qs == 128 * kc:  # diagonal block leads piece
                    if _mask_eng[0] is not None:
                        eng = _mask_eng[0]
                    else:
                        _alt[0] += 1
                        eng = (nc.vector.tensor_mul if _alt[0] % 2 == 0
                               else nc.gpsimd.tensor_mul)
                    eng(pt[:, 0:128], pt[:, 0:128], MSK)
                return pt, qs, cols

            def pv_piece(yaug, h, qh, kc, pt, qs, cols):
                # The diagonal (masked) 0:128 chunk is emitted LAST so the
                # unmasked bulk of PV never waits on the mask multiply.
                q0 = 1024 * qh
                diag = (qs == 128 * kc and cols > 128)
                off = 128 if diag else 0
                chunks = []
                while off < cols:
                    cw = min(512 - ((qs + off) % 512), cols - off)
                    chunks.append((off, cw))
                    off += cw
                if diag:
                    chunks.append((0, 128))
                started = set()
                for off, cw in chunks:
                    w = (qs + off) // 512
                    st = (kc == 0) and (w not in started)
                    if kc == 0:
                        started.add(w)
                    nc.tensor.matmul(
                        yaug[:, qs - q0 + off:qs - q0 + off + cw],
                        VA[:, kc, 65 * h:65 * h + 65],
                        pt[:, off:off + cw],
                        start=st, stop=(kc == 4 * w + 3))

            # ------- Phase 2: decoupled S/exp stream + lagged PV stream ----
            # The exp stream (Activation engine) is the global bottleneck:
            # S+exp pieces are emitted in one continuous stream (keeping the
            # scalar engine fed), while the PV/normalization consumer runs
            # LAG pieces behind, and the V projection weaves into the early
            # stream. pt tiles buffer the in-flight pieces.
            psy_cm = tc.tile_pool(name="psy", bufs=1, space="PSUM")
            p2_cm = tc.tile_pool(name="p2", bufs=2)
            p2 = p2_cm.__enter__()
            p4_cm = tc.tile_pool(name="p4", bufs=4)
            p4 = p4_cm.__enter__()
            rb_cm = tc.tile_pool(name="rb", bufs=2)
            rbp = rb_cm.__enter__()

            norm_state = {}
            LAG = 16

            def normalize(h, qh, yaug, last_pair):
                """Softmax denominator: stage, reciprocal, DRAM-bounce
                broadcast per head pair, then the normalizing muls."""
                q0, q1 = 1024 * qh, 1024 * qh + 1024
                hp = h // 2
                # Single stage-copy (65 rows incl. denominator) releases
                # the yaug PSUM buffer early. qh1 odd heads skip it (their
                # long period hides the bounce; mul reads PSUM directly).
                if qh == 0 or h % 2 == 0:
                    ystg = p4.tile([65, 1024], BF16, tag="ystg")
                    nc.vector.tensor_copy(out=ystg, in_=yaug)
                    den_src, y_src, y_psum = (ystg[64:65, :],
                                              ystg[0:64, :], 0)
                else:
                    den_src, y_src, y_psum = (yaug[64:65, :],
                                              yaug[0:64, :], 1)
                if h % 2 == 0:
                    rec2 = p4.tile([1, 2, 1024], BF16, tag="rec")
                    with nc.allow_low_precision(reason="rec bounce"):
                        nc.vector.reciprocal(out=rec2[:, 0, :], in_=den_src)
                    norm_state[(hp, qh)] = (rec2, y_src)
                    return
                rec2, ysrc_e = norm_state.pop((hp, qh))
                with nc.allow_low_precision(reason="rec bounce"):
                    nc.vector.reciprocal(out=rec2[:, 1, :], in_=den_src)
                wr_i = nc.sync.dma_start(
                    out=rec_d[hp, :, q0:q1],
                    in_=rec2.rearrange("p a c -> p (a c)"))
                rb2 = rbp.tile([64, 2, 1024], BF16, tag="rb")
                rsrc = rec_d[hp, 0, q0:q1]
                rd_i = nc.sync.dma_start(
                    out=rb2,
                    in_=bass.AP(tensor=rsrc.tensor, offset=rsrc.offset,
                                ap=[[0, 64], [T, 2]] + list(rsrc.ap)))
                # Tile does not track DRAM scratch RAW deps
                add_dep_helper(rd_i.ins, wr_i.ins, sync=True,
                               reason="recd bounce RAW")
                # even head -> AT rows 0:64 directly; odd staged + DMA.
                # Off the critical path these go to the Pool engine; the
                # final pair stays on DVE (2x bf16) to shorten the tail.
                meng = (nc.vector.tensor_mul if (last_pair or y_psum)
                        else nc.gpsimd.tensor_mul)
                meng2 = (nc.vector.tensor_mul if last_pair
                         else nc.gpsimd.tensor_mul)
                meng2(AT[hp][0:64, q0:q1], ysrc_e, rb2[:, 0, :])
                stg = p2.tile([64, 1024], BF16, tag="stg")
                meng(stg, y_src, rb2[:, 1, :])
                nc.sync.dma_start(out=AT[hp][64:128, q0:q1], in_=stg)

            head_order = [(0, 0), (1, 0), (2, 0), (3, 0),
                          (2, 1), (3, 1), (0, 1), (1, 1)]
            plist = [(h, qh, kc) for (h, qh) in head_order
                     for kc in range(8 * (qh + 1))]
            pieces = {}
            cons = {"i": 0, "yaug": None}

            def consume_one():
                h, qh, kc = plist[cons["i"]]
                cons["i"] += 1
                if kc == 0:
                    cons["yaug"] = psy.tile([65, 1024], F32, tag="yaug",
                                            name=f"ya{h}_{qh}")
                pv_piece(cons["yaug"], h, qh, kc, *pieces.pop((h, qh, kc)))
                if kc == 8 * (qh + 1) - 1:
                    normalize(h, qh, cons["yaug"], (h, qh) in
                              ((0, 1), (1, 1)))

            _mask_eng[0] = nc.gpsimd.tensor_mul
            vproj(0)
            vproj(1)
            psy = None
            for i, (h, qh, kc) in enumerate(plist):
                pieces[(h, qh, kc)] = s_exp_piece(h, qh, kc, ptp, "pt")
                if 1 <= i <= 14:
                    vproj(i + 1)
                if i == 14:
                    psv_cm.__exit__(None, None, None)
                    psy = psy_cm.__enter__()
                if i == 5:
                    rope(QT[1], "q1", 0, 1024)
                    rope(KT[1], "k1", 0, 1024)
                if i == 6:
                    for tb in range(4, 8):
                        xb_load(tb)
                if i == 24:
                    rope(QT[0], "q0b", 1024, T)
                    rope(KT[0], "k0b", 1024, T)
                    rope(QT[1], "q1b", 1024, T)
                    rope(KT[1], "k1b", 1024, T)
                if i == 63:
                    _mask_eng[0] = None
                if i >= LAG:
                    consume_one()
                if i >= len(plist) - 12 and cons["i"] < len(plist):
                    consume_one()

            # ---------------- Phase 3: output projection ----------------
            # Reuses the attention pools (sp tiles for PSUM accumulation,
            # pt tiles for the bf16 eviction staging) so no pool barrier
            # separates it from the final normalization.
            def oproj(t16):
                yo = pss.tile([128, D], F32, tag="sp", name=f"yo{t16}")
                for half in range(2):
                    for ft in (1, 0):
                        nc.tensor.matmul(
                            yo[:, 512 * half:512 * half + 512],
                            AT[ft][:, 128 * t16:128 * t16 + 128],
                            WOC[:, ft, 512 * half:512 * half + 512],
                            start=(ft == 1), stop=(ft == 0))
                return yo

            def oevict(t16, yo):
                ot = ptp.tile([128, D], BF16, tag="pt", name=f"ot{t16}")
                eng = (nc.scalar.copy, nc.vector.tensor_copy)[t16 % 2]
                eng(out=ot, in_=yo)
                deng = nc.sync if t16 % 2 == 0 else nc.scalar
                deng.dma_start(
                    out=yp_d[128 * t16:128 * t16 + 128, :], in_=ot)

            while cons["i"] < len(plist):
                consume_one()
            prev = None
            for t16 in range(16):
                yo = oproj(t16)
                if prev is not None:
                    oevict(t16 - 1, prev)
                prev = yo
            oevict(15, prev)

            psy_cm.__exit__(None, None, None)
            pss_cm.__exit__(None, None, None)
            rb_cm.__exit__(None, None, None)
            p4_cm.__exit__(None, None, None)
            p2_cm.__exit__(None, None, None)
            ptp_cm.__exit__(None, None, None)
            rp_cm.__exit__(None, None, None)

    _split_excess_waits(nc)
    return nc


_NC = None
_LAST_INMAPS = None


def _get_nc():
    global _NC
    if _NC is None:
        _NC = _build()
    return _NC


def _ternary_signs(w):
    """Mirror reference ternary_weight: returns (signs in {-1,0,1}, scale)."""
    try:
        import jax
        import jax.numpy as jnp
        cpu = jax.devices("cpu")[0]
        with jax.default_device(cpu):
            wj = jnp.asarray(np.asarray(w, dtype=np.float32))
            scale = jnp.mean(jnp.abs(wj))
            signs = jnp.round(jnp.clip(wj / (scale + 1e-8), -1.0, 1.0))
            return np.asarray(signs, dtype=np.float32), float(scale)
    except Exception:
        w = np.asarray(w, dtype=np.float32)
        scale = np.float32(np.mean(np.abs(w)))
        signs = np.round(np.clip(w / (scale + np.float32(1e-8)), -1.0, 1.0))
        return signs.astype(np.float32), float(scale)


def _rope_tables():
    inv = (1.0 / (10000.0 ** (np.arange(0, HD, 2, dtype=np.float32) / HD))
           ).astype(np.float32)                      # [32]
    t = np.arange(T, dtype=np.float32)
    fr = np.outer(t, inv).astype(np.float32)         # [T, 32]
    cos1 = np.cos(fr).astype(np.float32)             # [T, 32]
    sin1 = np.sin(fr).astype(np.float32)
    # rows: d in 0..63 (freq d%32), tiled for 2 heads -> 128 rows
    cosd = np.concatenate([cos1, cos1], axis=1).T    # [64, T]
    sind = np.concatenate([sin1, sin1], axis=1).T    # [64, T]
    sgn = np.ones((HD, 1), dtype=np.float32)
    sgn[:HD // 2] = -1.0
    cos2 = np.tile(cosd, (2, 1)).astype(np.float32)          # [128, T]
    sins = np.tile(sind * sgn, (2, 1)).astype(np.float32)    # [128, T]
    return cos2, sins


def kernel(x, Wq, Wk, Wv, Wo, mask):
    global _LAST_INMAPS
    import ml_dtypes
    F8 = ml_dtypes.float8_e4m3
    BF = ml_dtypes.bfloat16

    x = np.asarray(x, dtype=np.float32)
    mask = np.asarray(mask)
    assert np.array_equal(
        np.asarray(mask[0, 0], dtype=np.int32),
        np.tril(np.ones((T, T), dtype=np.int32))), "non-causal mask"

    qs, sq = _ternary_signs(Wq)
    ks, sk = _ternary_signs(Wk)
    vs, sv = _ternary_signs(Wv)
    os_, so = _ternary_signs(Wo)
    cos2, sins = _rope_tables()
    mvals = np.triu(np.ones((128, 128), dtype=np.float32))  # valid: k <= q
    consts = np.full((128, 1), np.float32(sq) * np.float32(sk) *
                     np.float32(0.125), dtype=np.float32)

    in_maps = []
    for c in range(NCORES):
        b, g = c // 4, c % 4
        fsl = slice(FPC * g, FPC * g + FPC)
        xt = np.ascontiguousarray(x[b].T)            # [D, T]
        # DR weight layout: w8[p, kcp, i, f] = signs[FPC*g+f, 256*kcp+128*i+p]
        wq8 = np.ascontiguousarray(
            qs[fsl].T.reshape(4, 2, 128, FPC).transpose(2, 0, 1, 3)
            ).reshape(128, 8 * FPC)
        wk8 = np.ascontiguousarray(
            ks[fsl].T.reshape(4, 2, 128, FPC).transpose(2, 0, 1, 3)
            ).reshape(128, 8 * FPC)
        wvb = np.ascontiguousarray(
            vs[fsl].T.reshape(KC, 128, FPC).transpose(1, 0, 2))
        wob = np.ascontiguousarray(
            (os_[:, fsl].T * np.float32(sv * so)).reshape(
                2, 128, D).transpose(1, 0, 2))
        in_maps.append({
            "x8": xt.astype(F8),
            "xb": xt.astype(BF),
            "wq8": wq8.astype(F8),
            "wk8": wk8.astype(F8),
            "wvb": wvb.astype(BF),
            "wob": wob.astype(BF),
            "cosb": cos2.astype(BF),
            "sinb": sins.astype(BF),
            "mskb": mvals.astype(BF),
            "conf": consts,
        })
    _LAST_INMAPS = in_maps

    res = run_bass_kernel_spmd(_get_nc(), in_maps,
                               core_ids=list(range(NCORES)))
    out = np.zeros((B, T, D), dtype=np.float32)
    for b in range(B):
        acc = np.zeros((T, D), dtype=np.float32)
        for g in range(4):
            acc += np.asarray(res.results[4 * b + g]["yp"],
                              dtype=np.float32)
        out[b] = acc
    return out


def bench(trace=True):
    """Re-run last inputs with NTFF tracing; returns BassKernelResults."""
    assert _LAST_INMAPS is not None, "call kernel() first"
    return run_bass_kernel_spmd(_get_nc(), _LAST_INMAPS,
                                core_ids=list(range(NCORES)), trace=trace)
